# revision 12
# baseline (speedup 1.0000x reference)
"""Trainium2 Bass kernel for a 2-layer LSTM (H=50) + linear head with
autoregressive future steps. Data-parallel over 8 NeuronCores (batch sharded).

Design "T" (T-form tanh-only activations, adjacent-block packing):
  - Per core 2048 samples: lo block (0:1024) on partition rows 0:50 of each
    gate/state group, hi block (1024:2048) on rows 50:100. Free dim = sample
    index within block; matmul passes of 512 (PSUM bank width).
  - Merged block-diagonal matmuls: one matmul per gate covers BOTH blocks
    (lhsT cols 0:50 = lo weights on lo rows, cols 50:100 = hi weights on hi
    rows, zeros elsewhere). 4 mms for cell1, 8 for cell2 (two accumulating
    streams) per 512-pass -> 24 mms/step vs 48 in the naive layout.
  - Everything nonlinear is Tanh (one ACT table, no reloads). Gate pre-acts
    for i,f,o are pre-scaled by 0.5 in the weights so T_x = tanh(pre/2) and
    sigmoid(pre) = (T_x+1)/2; the g gate is unscaled so T_g = tanh(pre_g).
  - State d = 2c; tanh(c) = ACT(Tanh, scale=0.5) on d. h' = 2h is stored and
    the 0.5 folded into all h-consuming weights. This makes every elementwise
    step a single fused DVE op:
      t = (T_i + 1)*T_g          [scalar_tensor_tensor]
      sf = T_f*0.5 + 0.5         [tensor_scalar]
      d  = sf*d_old + t          [2 ops]
      h' = (T_o + 1)*tanh(c)     [scalar_tensor_tensor]
  - y(t-1) = Wl h2(t-1) + bl rides the cell2 o-gate matmul as 2 extra lhsT
    columns (rows 100:102 of PSUM) and is DMA'd straight PSUM -> HBM fp32.
  - Future phase computes h2/y inline per step (y feeds back as x).
"""

import sys
import os
import numpy as np

for _p in ("/opt/trn_rl_repo", "/root/.axon_site/_ro/trn_rl_repo"):
    if os.path.isdir(_p) and _p not in sys.path:
        sys.path.insert(0, _p)
        break

from contextlib import ExitStack

import concourse.bass as bass
import concourse.mybir as mybir
import concourse.tile as tile
from concourse import bacc
from concourse.bass_utils import run_bass_kernel_spmd

FP16 = mybir.dt.float16
FP32 = mybir.dt.float32
AF = mybir.ActivationFunctionType
ALU = mybir.AluOpType

H = 50
B = 16384
NCORES = 8
BC = B // NCORES          # 2048 samples per core
HALF = 1024               # samples per partition-block (lo/hi)
PW = 512                  # matmul pass width (one PSUM bank of fp32)
SLAB = 102                # lhsT columns per gate slab (100 gates + 2 y cols)

# chunk order i, f, o, g ; torch block order: i, f, g, o
GATE_SRC = [0, 1, 3, 2]
GI, GF, GO, GG = 0, 1, 2, 3
YS = 8.0                  # y rides the tanh ACT scaled by 1/YS (~identity)


def _build_nc(T, FUT):
    TT = T + FUT
    nc = bacc.Bacc("TRN2", target_bir_lowering=False, debug=False,
                   num_devices=NCORES)

    xT = nc.dram_tensor("xT", [T, 2, HALF], FP16, kind="ExternalInput")
    W1M = nc.dram_tensor("W1M", [128, 4 * SLAB], FP16, kind="ExternalInput")
    W2A = nc.dram_tensor("W2A", [128, 4 * SLAB], FP16, kind="ExternalInput")
    W2B = nc.dram_tensor("W2B", [128, 4 * SLAB], FP16, kind="ExternalInput")
    WLY = nc.dram_tensor("WLY", [128, 2], FP16, kind="ExternalInput")
    ONES = nc.dram_tensor("ONES", [1, HALF], FP16, kind="ExternalInput")
    yT = nc.dram_tensor("yT", [TT, 2, HALF], FP16, kind="ExternalOutput")
    DUM = nc.dram_tensor("DUM", [128, 32], FP16, kind="ExternalOutput")

    with tile.TileContext(nc) as tc, ExitStack() as ctx:
        const = ctx.enter_context(tc.tile_pool(name="const", bufs=1))
        state = ctx.enter_context(tc.tile_pool(name="state", bufs=1))
        ta_p = ctx.enter_context(tc.tile_pool(name="ta", bufs=2))
        tb_p = ctx.enter_context(tc.tile_pool(name="tb", bufs=3))
        tc_p = ctx.enter_context(tc.tile_pool(name="tcp", bufs=2))
        tmp_p = ctx.enter_context(tc.tile_pool(name="tmp", bufs=10))
        y_p = ctx.enter_context(tc.tile_pool(name="yp", bufs=3))
        pg_p = ctx.enter_context(tc.tile_pool(name="pg", bufs=2, space="PSUM"))

        w1m = const.tile([128, 4 * SLAB], FP16, tag="w1m")
        w2a = const.tile([128, 4 * SLAB], FP16, tag="w2a")
        w2b = const.tile([128, 4 * SLAB], FP16, tag="w2b")
        wly = const.tile([128, 2], FP16, tag="wly")
        nc.sync.dma_start(out=w1m[:], in_=W1M.ap())
        nc.sync.dma_start(out=w2a[:], in_=W2A.ap())
        nc.sync.dma_start(out=w2b[:], in_=W2B.ap())
        nc.sync.dma_start(out=wly[:], in_=WLY.ap())

        # S1: rows 0:100 h1' (lo 0:50, hi 50:100), 100 x-lo, 101 x-hi, 102 one
        s1 = [state.tile([128, HALF], FP16, tag=f"s1_{b}", name=f"s1_{b}")
              for b in range(2)]
        # S2: rows 0:100 h2', 100 one
        s2 = [state.tile([128, HALF], FP16, tag=f"s2_{b}", name=f"s2_{b}")
              for b in range(2)]
        # D state (= 2c), free dims (pass, 1024): [..., 0:512]=d1, [...,512:]=d2
        dst = state.tile([128, 2, HALF], FP16, tag="dst")

        for b in range(2):
            nc.vector.memset(s1[b][:], 0.0)
            nc.vector.memset(s2[b][:], 0.0)
            nc.sync.dma_start(out=s1[b][102:103, :], in_=ONES.ap())
            nc.sync.dma_start(out=s2[b][100:101, :], in_=ONES.ap())
        nc.vector.memset(dst[:], 0.0)

        def dma_x_in(t):
            sl = s1[t % 2]
            nc.sync.dma_start(out=sl[100:102, :], in_=xT.ap()[t:t + 1, :, :])

        dma_x_in(0)
        if T > 1:
            dma_x_in(1)

        def gv(tile_, G):
            # gate-G view across both passes: [100, 2, 512]
            return tile_[0:100, :, G * PW:(G + 1) * PW]

        tb_prev = None
        s2c, s2n = s2[0], s2[1]

        for t in range(TT):
            S1c, S1n = s1[t % 2], s1[(t + 1) % 2]
            future = t >= T - 1

            # ---- cell1 matmuls ----
            pg1 = [None, None]
            for p in range(2):
                fs = bass.ds(p * PW, PW)
                pg = pg_p.tile([128, 2048], FP32, tag="pg", name=f"pg1_{p}")
                for G in range(4):
                    nc.tensor.matmul(pg[0:102, G * PW:(G + 1) * PW],
                                     w1m[0:103, G * SLAB:G * SLAB + SLAB],
                                     S1c[0:103, fs], start=True, stop=True)
                pg1[p] = pg

            ta = ta_p.tile([128, 2, 2048], FP16, tag="ta", name="ta")
            for p in range(2):
                nc.scalar.activation(ta[0:102, p:p + 1, :], pg1[p][0:102, :],
                                     AF.Tanh)

            # ---- cell1 elementwise: d1 = sf*d1 + (Ti+1)*Tg ----
            t1 = tmp_p.tile([128, 2, PW], FP16, tag="t1", name="t1")
            nc.vector.scalar_tensor_tensor(t1[0:100, :, :], gv(ta, GI), 1.0,
                                           gv(ta, GG), ALU.add, ALU.mult)
            sf1 = tmp_p.tile([128, 2, PW], FP16, tag="sf1", name="sf1")
            nc.vector.tensor_scalar(sf1[0:100, :, :], gv(ta, GF), 0.5, 0.5,
                                    ALU.mult, ALU.add)
            u1 = tmp_p.tile([128, 2, PW], FP16, tag="u1", name="u1")
            nc.vector.tensor_mul(u1[0:100, :, :], sf1[0:100, :, :],
                                 dst[0:100, :, 0:PW])
            nc.vector.tensor_add(dst[0:100, :, 0:PW], t1[0:100, :, :],
                                 u1[0:100, :, :])

            # ---- tanh(c) for both cells in one ACT op (scale=0.5 on d) ----
            tct = tc_p.tile([128, 2, HALF], FP16, tag="tct", name="tct")
            nc.scalar.activation(tct[0:100, :, :], dst[0:100, :, :],
                                 AF.Tanh, scale=0.5)

            # h1' = (To+1)*tanh(c1) -> S1n rows 0:100
            nc.vector.scalar_tensor_tensor(S1n[0:100, :], gv(ta, GO), 1.0,
                                           tct[0:100, :, 0:PW],
                                           ALU.add, ALU.mult)
            # h2'(t-1) = (To2+1)*tanh(c2(t-1)) -> s2c rows 0:100
            if 0 < t < T:
                nc.vector.scalar_tensor_tensor(s2c[0:100, :],
                                               gv(tb_prev, GO), 1.0,
                                               tct[0:100, :, PW:2 * PW],
                                               ALU.add, ALU.mult)

            # ---- cell2 matmuls (two accumulating streams) + y(t-1) ----
            pg2 = [None, None]
            for p in range(2):
                fs = bass.ds(p * PW, PW)
                pg = pg_p.tile([128, 2048], FP32, tag="pg", name=f"pg2_{p}")
                for G in range(4):
                    sl = bass.ds(G * SLAB, SLAB)
                    nc.tensor.matmul(pg[0:102, G * PW:(G + 1) * PW],
                                     w2a[0:103, sl], S1n[0:103, fs],
                                     start=True, stop=False)
                    nc.tensor.matmul(pg[0:102, G * PW:(G + 1) * PW],
                                     w2b[0:101, sl], s2c[0:101, fs],
                                     start=False, stop=True)
                pg2[p] = pg

            tb = tb_p.tile([128, 2, 2048], FP16, tag="tb", name="tb")
            for p in range(2):
                nc.scalar.activation(tb[0:102, p:p + 1, :], pg2[p][0:102, :],
                                     AF.Tanh)
            if 0 < t < T:
                # y(t-1)/YS rode the o-chunk as rows 100:102; the tb tanh is
                # ~identity there (|y/YS| < 0.1), recover y with one DVE op.
                ys = y_p.tile([128, HALF], FP16, tag="ys", name="ys")
                nc.vector.tensor_scalar(ys[0:32, :],
                                        tb[96:128, :, GO * PW:(GO + 1) * PW],
                                        float(YS), 0.0, ALU.mult, ALU.add)
                nc.sync.dma_start(out=yT.ap()[t - 1:t, :, :], in_=ys[4:6, :])

            # ---- cell2 elementwise: d2 = sf2*d2 + (Ti2+1)*Tg2 ----
            t2 = tmp_p.tile([128, 2, PW], FP16, tag="t2", name="t2")
            nc.vector.scalar_tensor_tensor(t2[0:100, :, :], gv(tb, GI), 1.0,
                                           gv(tb, GG), ALU.add, ALU.mult)
            sf2 = tmp_p.tile([128, 2, PW], FP16, tag="sf2", name="sf2")
            nc.vector.tensor_scalar(sf2[0:100, :, :], gv(tb, GF), 0.5, 0.5,
                                    ALU.mult, ALU.add)
            u2 = tmp_p.tile([128, 2, PW], FP16, tag="u2", name="u2")
            nc.vector.tensor_mul(u2[0:100, :, :], sf2[0:100, :, :],
                                 dst[0:100, :, PW:2 * PW])
            nc.vector.tensor_add(dst[0:100, :, PW:2 * PW], t2[0:100, :, :],
                                 u2[0:100, :, :])

            if future:
                # inline h2'(t), y(t); y feeds x(t+1)
                tcf = tc_p.tile([128, 2, PW], FP16, tag="tcf", name="tcf")
                nc.scalar.activation(tcf[0:100, :, :],
                                     dst[0:100, :, PW:2 * PW],
                                     AF.Tanh, scale=0.5)
                nc.vector.scalar_tensor_tensor(s2n[0:100, :], gv(tb, GO), 1.0,
                                               tcf[0:100, :, :],
                                               ALU.add, ALU.mult)
                pgy = pg_p.tile([128, HALF], FP32, tag="pg", name="pgy")
                for p in range(2):
                    fs = bass.ds(p * PW, PW)
                    nc.tensor.matmul(pgy[0:2, fs], wly[0:101, :],
                                     s2n[0:101, fs], start=True, stop=True)
                ysf = y_p.tile([128, HALF], FP16, tag="ys", name="ysf")
                nc.vector.tensor_copy(ysf[0:2, :], pgy[0:2, :])
                nc.sync.dma_start(out=yT.ap()[t:t + 1, :, :], in_=ysf[0:2, :])
                if t + 1 < TT:
                    nc.sync.dma_start(out=S1n[100:102, :], in_=ysf[0:2, :])
                s2c, s2n = s2n, s2c
            tb_prev = tb
            if t + 2 < T:
                dma_x_in(t + 2)

        dum = tmp_p.tile([128, 32], FP16, tag="dum", name="dum")
        nc.vector.memset(dum[:], 0.0)
        nc.sync.dma_start(out=DUM.ap(), in_=dum[:])

    nc.compile()
    return nc


def _prep_weights(Wih1, Whh1, bih1, bhh1, Wih2, Whh2, bih2, bhh2, Wl, bl):
    b1 = (bih1 + bhh1).astype(np.float32)
    b2 = (bih2 + bhh2).astype(np.float32)

    W1M = np.zeros((128, 4 * SLAB), np.float32)
    W2A = np.zeros((128, 4 * SLAB), np.float32)
    W2B = np.zeros((128, 4 * SLAB), np.float32)
    WLY = np.zeros((128, 2), np.float32)
    for G, src in enumerate(GATE_SRC):
        blk = slice(src * H, (src + 1) * H)
        gs = 1.0 if G == GG else 0.5      # T-form gate scale
        hs = gs * 0.5                     # h' = 2h fold
        c0 = G * SLAB
        lo, hi = slice(c0, c0 + 50), slice(c0 + 50, c0 + 100)
        W1M[0:50, lo] = Whh1[blk, :].T * hs
        W1M[50:100, hi] = Whh1[blk, :].T * hs
        W1M[100, lo] = Wih1[blk, 0] * gs
        W1M[101, hi] = Wih1[blk, 0] * gs
        W1M[102, lo] = b1[blk] * gs
        W1M[102, hi] = b1[blk] * gs
        W2A[0:50, lo] = Wih2[blk, :].T * hs
        W2A[50:100, hi] = Wih2[blk, :].T * hs
        W2B[0:50, lo] = Whh2[blk, :].T * hs
        W2B[50:100, hi] = Whh2[blk, :].T * hs
        W2B[100, lo] = b2[blk] * gs
        W2B[100, hi] = b2[blk] * gs
    # y columns in the o slab: pre-act = (Wl*0.5 @ h2' + bl)/YS so the tb
    # tanh is ~identity on it; recovered by *YS on DVE.
    c0 = GO * SLAB
    W2B[0:50, c0 + 100] = Wl[0, :] * 0.5 / YS
    W2B[100, c0 + 100] = bl[0] / YS
    W2B[50:100, c0 + 101] = Wl[0, :] * 0.5 / YS
    W2B[100, c0 + 101] = bl[0] / YS
    WLY[0:50, 0] = Wl[0, :] * 0.5
    WLY[100, 0] = bl[0]
    WLY[50:100, 1] = Wl[0, :] * 0.5
    WLY[100, 1] = bl[0]
    return (W1M.astype(np.float16), W2A.astype(np.float16),
            W2B.astype(np.float16), WLY.astype(np.float16))


_NC_CACHE = {}
_last_in_maps = None


def _run(x, Wih1, Whh1, bih1, bhh1, Wih2, Whh2, bih2, bhh2, Wl, bl, future,
         trace=False):
    x = np.asarray(x, np.float32)
    nB, T = x.shape
    FUT = int(future)
    assert nB == B, (nB, B)

    key = (T, FUT)
    if key not in _NC_CACHE:
        _NC_CACHE[key] = _build_nc(T, FUT)
    nc = _NC_CACHE[key]

    W1M, W2A, W2B, WLYa = _prep_weights(
        np.asarray(Wih1, np.float32), np.asarray(Whh1, np.float32),
        np.asarray(bih1, np.float32), np.asarray(bhh1, np.float32),
        np.asarray(Wih2, np.float32), np.asarray(Whh2, np.float32),
        np.asarray(bih2, np.float32), np.asarray(bhh2, np.float32),
        np.asarray(Wl, np.float32), np.asarray(bl, np.float32))

    in_maps = []
    for c in range(NCORES):
        xc = np.ascontiguousarray(
            x[c * BC:(c + 1) * BC, :].T).reshape(T, 2, HALF).astype(np.float16)
        in_maps.append({"xT": xc, "W1M": W1M, "W2A": W2A, "W2B": W2B,
                        "WLY": WLYa, "ONES": np.ones((1, HALF), np.float16)})

    global _last_in_maps
    _last_in_maps = in_maps
    res = run_bass_kernel_spmd(nc, in_maps, list(range(NCORES)), trace=trace)
    out = np.empty((B, T + FUT), np.float32)
    for c in range(NCORES):
        yc = res.results[c]["yT"].astype(np.float32)  # [TT, 2, HALF]
        out[c * BC:c * BC + HALF, :] = yc[:, 0, :].T
        out[c * BC + HALF:(c + 1) * BC, :] = yc[:, 1, :].T
    return out, res


def kernel(**inputs):
    out, _ = _run(**inputs)
    return out
